# revision 6
# baseline (speedup 1.0000x reference)
"""EWMA predictor (sliding-window variance, exponentially weighted sum) on 8 trn2 cores.

Math: for j in [0, L): window_j = x[j : j+128], weight ff^(L-1-j),
result = norm * sum_j ff^(L-1-j) * var(window_j, ddof=1),
norm = (1-ff)/(1-ff^L), ff = sigmoid(raw_forgetting_factor).

Sharding: windows split over 8 cores x 128 partitions; partition p of core c
owns the 512 consecutive windows starting at base_c + 512*p and loads the 639
input elements covering them (halo overlap, contiguous per partition). The
per-core input tile carries ff in an extra trailing column, so one DMA loads
everything.

Per-core device program (vector + scalar engines):
  s1[t], s2[t]: sliding 128-window sums of x and x^2 via tensor_tensor_scan
                recurrence s[t] = (x[t+127] + s[t-1]) - x[t-1]
  d[t] = s2 - s1^2/128 = 127 * var
  e[t] = ff*e[t-1] + d[t]  (scan, ff read via stride-0 broadcast AP)
  => e[511] = sum_t ff^(511-t) d[t]
Host folds per-partition factors ff^(i0)/127 and norm in float64.
"""

import numpy as np

import concourse.bass as bass
import concourse.mybir as mybir
from concourse.bass_utils import run_bass_kernel_spmd

L = 524288          # look-back windows
W = 128             # variance window length
N = L + W           # input length
NCORES = 8
WIN_PER_CORE = L // NCORES      # 65536
RUN = WIN_PER_CORE // 128       # 512 windows per partition
COLS = RUN + W - 1              # 639 input elems per partition

_NC_CACHE = {}


def build_nc() -> bass.Bass:
    nc = bass.Bass(trn_type="TRN2")
    f32 = mybir.dt.float32
    A = mybir.AluOpType
    xt = nc.declare_dram_parameter("xt", [128, COLS + 1], f32, isOutput=False)
    acc = nc.declare_dram_parameter("acc", [128, 1], f32, isOutput=True)

    with (
        nc.sbuf_tensor([128, COLS + 1], f32) as XT,
        nc.sbuf_tensor([128, COLS], f32) as X2,
        nc.sbuf_tensor([128, RUN], f32) as S1,
        nc.sbuf_tensor([128, RUN], f32) as S2,
        nc.sbuf_tensor([128, RUN], f32) as T2,
        nc.sbuf_tensor([128, RUN], f32) as D,
        nc.sbuf_tensor([128, RUN], f32) as E,
        nc.sbuf_tensor([128, 1], f32) as WU2,
        nc.semaphore() as dsem,
        nc.semaphore() as vsem,
        nc.semaphore() as ssem,
        nc.Block() as block,
    ):

        HALF = (COLS + 1) // 2  # 320: column split point for the two DMA rings

        @block.sync
        def _(sync):
            sync.dma_start(XT[:, 0:HALF], xt[:, 0:HALF]).then_inc(dsem, 16)
            sync.wait_ge(vsem, 6)
            sync.dma_start(acc[:], E[:, RUN - 1 : RUN]).then_inc(dsem, 16)
            sync.wait_ge(dsem, 48)

        @block.scalar
        def _(scalar):
            # second input half on the ACT HWDGE ring, parallel with the SP ring
            scalar.dma_start(XT[:, HALF : COLS + 1], xt[:, HALF : COLS + 1]).then_inc(
                dsem, 16
            )
            # warmup: pull the activation-table load off the critical path
            # (runs during the input DMA; result never read)
            scalar.square(WU2[:], nc.const_aps.tensor(0.0, (128, 1)))
            scalar.wait_ge(dsem, 32)
            scalar.square(X2[:], XT[:, 0:COLS]).then_inc(ssem, 1)
            scalar.wait_ge(vsem, 2)
            scalar.square(T2[:], S1[:]).then_inc(ssem, 1)

        @block.vector
        def _(vector):
            vector.wait_ge(dsem, 32)
            vector.reduce_sum(
                S1[:, 0:1], XT[:, 0:W], axis=mybir.AxisListType.X
            ).then_inc(vsem, 1)
            vector.wait_ge(vsem, 1)  # RAW: scan initial reads S1[:,0:1]
            vector.tensor_tensor_scan(
                S1[:, 1:RUN],
                XT[:, W:COLS],
                XT[:, 0 : RUN - 1],
                initial=S1[:, 0:1],
                op0=A.add,
                op1=A.subtract,
            ).then_inc(vsem, 1)  # vsem=2
            vector.wait_ge(ssem, 1)
            vector.reduce_sum(
                S2[:, 0:1], X2[:, 0:W], axis=mybir.AxisListType.X
            ).then_inc(vsem, 1)  # vsem=3
            vector.wait_ge(vsem, 3)  # RAW: scan initial reads S2[:,0:1]
            vector.tensor_tensor_scan(
                S2[:, 1:RUN],
                X2[:, W:COLS],
                X2[:, 0 : RUN - 1],
                initial=S2[:, 0:1],
                op0=A.add,
                op1=A.subtract,
            ).then_inc(vsem, 1)  # vsem=4
            vector.wait_ge(ssem, 2)
            vector.wait_ge(vsem, 4)  # RAW: D reads S2 written by prior scan
            vector.scalar_tensor_tensor(
                D[:], T2[:], -1.0 / 128.0, S2[:], op0=A.mult, op1=A.add
            ).then_inc(vsem, 1)  # vsem=5
            vector.wait_ge(vsem, 5)  # RAW: E-scan reads D
            vector.tensor_tensor_scan(
                E[:],
                XT[:, COLS : COLS + 1].broadcast_to([128, RUN]),
                D[:],
                initial=0.0,
                op0=A.mult,
                op1=A.add,
            ).then_inc(vsem, 1)  # vsem=6

    return nc


def _get_nc() -> bass.Bass:
    if "nc" not in _NC_CACHE:
        _NC_CACHE["nc"] = build_nc()
    return _NC_CACHE["nc"]


def make_in_maps(x: np.ndarray, ff32: np.float32) -> list[dict[str, np.ndarray]]:
    in_maps = []
    for c in range(NCORES):
        base = c * WIN_PER_CORE
        xt = np.empty((128, COLS + 1), dtype=np.float32)
        xt[:, 0:COLS] = np.lib.stride_tricks.as_strided(
            x[base:], shape=(128, COLS), strides=(RUN * 4, 4)
        )
        xt[:, COLS] = ff32
        in_maps.append({"xt": xt})
    return in_maps


def combine_host(accs: list[np.ndarray], ff32: np.float32) -> np.ndarray:
    """accs: per-core [128,1] device partials. Float64 host reduction."""
    ff64 = np.float64(ff32)
    lnff = np.log(ff64)
    p = np.arange(128)
    total = np.float64(0.0)
    for c in range(NCORES):
        e = np.asarray(accs[c]).reshape(128).astype(np.float64)
        i0 = L - 1 - (c * WIN_PER_CORE + RUN * p + (RUN - 1))
        total += np.sum(np.exp(lnff * i0) / 127.0 * e)
    norm = (1.0 - ff64) / (1.0 - np.exp(lnff * L))
    return np.asarray(np.float32(norm * total))


def kernel(past_returns, features, raw_forgetting_factor):
    x = np.ascontiguousarray(np.asarray(past_returns, dtype=np.float32))
    assert x.shape == (N,), x.shape
    raw = np.float64(np.asarray(raw_forgetting_factor).reshape(-1)[0])
    ff32 = np.float32(1.0 / (1.0 + np.exp(-raw)))

    nc = _get_nc()
    in_maps = make_in_maps(x, ff32)
    res = run_bass_kernel_spmd(nc, in_maps, list(range(NCORES)))
    accs = [res.results[c]["acc"] for c in range(NCORES)]
    return combine_host(accs, ff32)
